# revision 16
# baseline (speedup 1.0000x reference)
"""GCNConv + PReLU + L2-normalize + global_mean_pool on 8 trn2 NeuronCores.

Strategy: edge-parallel with host-materialized messages.  All gather
indices are host-known, so instead of an on-device row gather (the
original design was bound by SWDGE gather-descriptor generation on
GPSIMD at ~5ns/256B-row), the host pre-gathers dinv[s]*x[s] for every
edge slot into a dense, destination-tile-grouped chunk stream.
Self-loops are ordinary edges (s == d) under the identity

  out[d] = dinv[d] * ( sum_{s->d, s incl. d} dinv[s]*x[s] ) @ W + b

because W is linear and factors out of the edge sum.  The device work
per destination tile t (128 nodes) is then:

  1. bulk-stream the tile group's chunk block Xc [128 slots, ~36KB]
  2. one-hot OH[slot, dst] built on DVE from packed dst locators
  3. KE scatter matmuls accumulating accT[feat, dst] in PSUM
     (lhsT = Xc chunk [slot, feat], rhs = OH chunk [slot, dst])
  4. one W matmul: u[dst, h] = accT^T @ W  (accT is already [feat, dst],
     exactly the lhsT the PE wants -- no transposes anywhere)
  5. epilogue: *dinv[d], +b, PReLU, L2-normalize, pooling matmul into a
     per-graph PSUM accumulator

Per-graph partial sums are AllReduced across the 8 cores and divided by
per-graph node counts.  No AllGather, no phase-1 x@W, no per-edge
indexed DMA.

DMA mechanism (measured): a plain dma_start drains its descriptors on a
single DMA engine (~21 GB/s/ring; only the SP and ACT HWDGE rings
exist), which would make the ~55 MB/core stream take ~2.6 ms.
dma_gather with single_packet=False instead spreads per-index packets
across all DMA engines, so the stream is fetched by dma_gather calls of
128 host-baked *sequential* indices, each index naming one ~36KB
[partition x tile-group] row of the host-packed stream: ~1.7us of
descriptor generation per ~4.6MB call, drain at the full HBM rate.

Nodes are assigned to (core, tile, partition) snake-balanced by
in-degree so per-tile chunk counts are uniform across cores (the chunk
count per tile is baked into the shared SPMD program as the max over
cores).  Padding slots carry zero rows and a dst locator of 255 which
never matches the 0..127 iota; padded node rows carry batch id 255 so
the pooling one-hot drops them.

Roofline: ~212k edge slots/core * 256B (bf16) = ~55 MB of sequential
HBM reads = ~155us at 358 GB/s, with PE (~110us) and DVE (~90us)
hidden underneath.
"""

import numpy as np
import ml_dtypes

import concourse.bacc as bacc
import concourse.tile as tile
import concourse.mybir as mybir
from concourse.bass_utils import run_bass_kernel_spmd

P = 128          # partitions / tile size
D = 128          # feature dim
G = 128          # number of graphs
NCORES = 8
TGRP = 8         # dst tiles per gather group
EB = 2048        # gather piece size in bytes (one DMA packet per index)
PIECE = EB // 256            # chunks per piece (bf16 chunk row = 256B)
MAXIDX = 8 * P   # max indices per dma_gather call (validated on HW)
MODE = "tgather"             # "tgather" | "gather" | "dma"
TEB = 8192       # transpose-gather row bytes (-> 32-col x 128-part tile)
TCH = TEB // 256             # chunks covered per transpose-gather index
TEL = TEB // 2               # elements per transpose-gather row (bf16)

F32 = mybir.dt.float32
BF16 = mybir.dt.bfloat16
I16 = mybir.dt.int16
AF = mybir.ActivationFunctionType
OP = mybir.AluOpType

XDT = BF16
XDT_NP = ml_dtypes.bfloat16


# ----------------------------------------------------------------------------
# Host-side packing: node permutation, edge->slot layout, per-core arrays.
# ----------------------------------------------------------------------------
def pack_inputs(x, edge_index, batch):
    N, Dx = x.shape
    E = edge_index.shape[1]
    src0 = edge_index[0].astype(np.int64)
    dst0 = edge_index[1].astype(np.int64)

    ntiles = -(-(-(-N // P)) // NCORES) * NCORES
    TPC = ntiles // NCORES
    NPC = TPC * P
    NPAD = NCORES * NPC

    indeg = np.bincount(dst0, minlength=N).astype(np.int64)
    dinv = (1.0 / np.sqrt((indeg + 1).astype(np.float64))).astype(np.float32)

    # ---- assign nodes to tiles: snake over tiles in descending in-degree ----
    order = np.argsort(-indeg, kind="stable")
    nrounds = NPAD // ntiles
    tile_seq = np.arange(ntiles)
    snake = np.empty((nrounds, ntiles), np.int64)
    snake[0::2] = tile_seq
    snake[1::2] = tile_seq[::-1]
    tile_of_slot = snake.reshape(-1)          # [NPAD]
    p_of_slot = np.repeat(np.arange(nrounds), ntiles)
    node_of_slot = np.full(NPAD, -1, np.int64)
    node_of_slot[:N] = order

    load = np.zeros(ntiles, np.int64)
    np.add.at(load, tile_of_slot[:N], indeg[order] + 1)

    # ---- assign tiles to cores: snake over cores in descending load ----
    tord = np.argsort(-load, kind="stable")
    core_of_tile = np.empty(ntiles, np.int64)
    tidx_of_tile = np.empty(ntiles, np.int64)
    cseq = np.arange(NCORES)
    for r in range(TPC):
        cs = cseq if r % 2 == 0 else cseq[::-1]
        tr = tord[r * NCORES:(r + 1) * NCORES]
        core_of_tile[tr] = cs
        tidx_of_tile[tr] = r

    row_of_slot = (core_of_tile[tile_of_slot] * NPC
                   + tidx_of_tile[tile_of_slot] * P + p_of_slot)
    row_of_node = np.empty(N, np.int64)
    real = node_of_slot >= 0
    row_of_node[node_of_slot[real]] = row_of_slot[real]
    node_at_row = np.full(NPAD, -1, np.int64)
    node_at_row[row_of_slot] = node_of_slot

    # ---- edge slots (self-loops appended as ordinary edges) ----
    loop = np.arange(N, dtype=np.int64)
    src = np.concatenate([src0, loop])
    dst = np.concatenate([dst0, loop])
    er = row_of_node[dst]
    ec = er // NPC
    et = (er % NPC) // P
    ep = er % P
    gt = ec * TPC + et
    eo = np.argsort(gt, kind="stable")
    gts = gt[eo]
    srcs = src[eo]
    eps = ep[eo]

    mct = np.bincount(gt, minlength=ntiles).reshape(NCORES, TPC)
    KE = np.maximum(1, -(-mct.max(axis=0) // P)).astype(np.int64)  # [TPC]
    CB = np.concatenate([[0], np.cumsum(KE)]).astype(np.int64)

    # ---- gather groups of TGRP tiles, padded to uniform width GW chunks ----
    NG = -(-TPC // TGRP)
    gwidths = [int(CB[min((g + 1) * TGRP, TPC)] - CB[g * TGRP])
               for g in range(NG)]
    align = TCH if MODE == "tgather" else PIECE
    GW = -(-max(gwidths) // align) * align
    CHKP = NG * GW                       # padded chunk count

    starts = np.searchsorted(gts, np.arange(ntiles))
    j = np.arange(E + N) - starts[gts]
    slotp = j % P
    tloc = gts % TPC                     # tile index within core
    grp = tloc // TGRP
    # chunk column in the padded group-local layout
    colp = grp * GW + (CB[tloc] - CB[grp * TGRP]) + j // P
    core = gts // TPC

    xs = (np.asarray(x, np.float32) * dinv[:, None]).astype(XDT_NP)
    if MODE == "tgather":
        # table row (q*128 + i) is call q's index i; its TEL elements land
        # transposed as the [128-partition x (TCH*128-col)/128-col] tile:
        # elem[j*128 + c] -> stream[part c, call-base + j*128 + i]
        NQ = NG * (GW // TCH)
        NROWS = NQ * P
        assert NROWS <= 32767
        xth = np.zeros((NCORES, P, CHKP, Dx), XDT_NP)
        xth[core, slotp, colp] = xs[srcs]
        xcg = np.ascontiguousarray(
            xth.reshape(NCORES, P, NQ, TEL // P, P)
            .transpose(0, 2, 4, 3, 1)).reshape(NCORES, NROWS, TEL)
        del xth
    else:
        # xcg row (q*128 + p) is piece q (PIECE chunks) of partition p's
        # stream; piece q = g*GP + l covers group g's local chunks
        # [l*PIECE, (l+1)*PIECE)
        GP = GW // PIECE
        NROWS = NG * GP * P
        assert NROWS <= 32767
        lc = colp - grp * GW             # local chunk within group
        xcg = np.zeros((NCORES, NROWS, PIECE, Dx), XDT_NP)
        xcg[core, (grp * GP + lc // PIECE) * P + slotp,
            lc % PIECE] = xs[srcs]
        xcg = xcg.reshape(NCORES, NROWS, PIECE * Dx)
    dstloc = np.full((NCORES, P, CHKP), 255.0, ml_dtypes.bfloat16)
    dstloc[core, slotp, colp] = eps.astype(ml_dtypes.bfloat16)

    # gather indices are the sequence 0..NROWS-1 in the wrapped int16 layout
    # (value i lives at [i%16, i//16], replicated to 128 partitions)
    idx16 = np.zeros((16, NROWS // 16), np.int16)
    i = np.arange(NROWS)
    idx16[i % 16, i // 16] = i.astype(np.int16)
    idx16 = np.tile(idx16, (8, 1))

    # ---- per-core node arrays ----
    nar = node_at_row.reshape(NCORES, NPC)
    dinvsh = np.ones((NCORES, P, TPC), np.float32)
    batsh = np.full((NCORES, P, TPC), 255.0, np.float32)
    for c in range(NCORES):
        m = nar[c] >= 0
        dv = np.ones(NPC, np.float32)
        dv[m] = dinv[nar[c][m]]
        dinvsh[c] = dv.reshape(TPC, P).T
        bt = np.full(NPC, 255.0, np.float32)
        bt[m] = batch[nar[c][m]].astype(np.float32)
        batsh[c] = bt.reshape(TPC, P).T
    batsh = batsh.astype(ml_dtypes.bfloat16)

    cnt = np.bincount(np.asarray(batch, np.int64), minlength=G)
    cnt = cnt.astype(np.float32).reshape(G, 1)

    return dict(TPC=TPC, KE=KE, GW=GW, NG=NG,
                xcg=xcg, dstloc=dstloc, idx16=idx16,
                dinvsh=dinvsh, batsh=batsh, cnt=cnt)


# ----------------------------------------------------------------------------
# Device program.
# ----------------------------------------------------------------------------
def build_program(TPC, KE, GW, NG, repeats=1, mode=MODE):
    KE = [int(k) for k in KE]
    CB = np.concatenate([[0], np.cumsum(KE)]).astype(np.int64)
    KMAX = max(KE)
    CHKP = NG * GW
    if mode == "tgather":
        CPG = GW // TCH                  # transpose-gather calls per group
        NROWS = NG * CPG * P
        ROWEL = TEL
    else:
        GP = GW // PIECE
        NROWS = NG * GP * P
        ROWEL = PIECE * D
        # per-group gather calls: (local piece base, piece count)
        calls = [(l0, min(MAXIDX // P, GP - l0))
                 for l0 in range(0, GP, MAXIDX // P)]

    nc = bacc.Bacc("TRN2", target_bir_lowering=False, debug=False,
                   num_devices=NCORES)

    xcg_in = nc.dram_tensor("xcg", [NROWS, ROWEL], XDT,
                            kind="ExternalInput")
    idx_in = nc.dram_tensor("idx16", [P, NROWS // 16], I16,
                            kind="ExternalInput")
    dst_in = nc.dram_tensor("dstloc", [P, CHKP], BF16, kind="ExternalInput")
    dinv_in = nc.dram_tensor("dinvsh", [P, TPC], F32, kind="ExternalInput")
    bat_in = nc.dram_tensor("batsh", [P, TPC], BF16, kind="ExternalInput")
    w_in = nc.dram_tensor("w", [D, D], F32, kind="ExternalInput")
    b_in = nc.dram_tensor("b", [1, D], F32, kind="ExternalInput")
    a_in = nc.dram_tensor("a", [1, D], F32, kind="ExternalInput")
    cnt_in = nc.dram_tensor("cnt", [G, 1], F32, kind="ExternalInput")
    pooled_out = nc.dram_tensor("pooled", [G, D], F32, kind="ExternalOutput")

    ar_in = nc.dram_tensor("ar_in", [G, D], F32)
    ar_out = nc.dram_tensor("ar_out", [G, D], F32, addr_space="Shared")

    with tile.TileContext(nc, num_cores=NCORES) as tc:
        with (
            tc.tile_pool(name="const", bufs=1) as constp,
            tc.tile_pool(name="meta", bufs=1) as metap,
        ):
            # ---- constants ----
            w_t = constp.tile([D, D], F32)
            nc.sync.dma_start(out=w_t[:], in_=w_in[:])
            brow = constp.tile([1, D], F32)
            nc.sync.dma_start(out=brow[:], in_=b_in[:])
            bbc = constp.tile([P, D], F32)
            nc.gpsimd.partition_broadcast(bbc[:], brow[:])
            arow = constp.tile([1, D], F32)
            nc.sync.dma_start(out=arow[:], in_=a_in[:])
            abc = constp.tile([P, D], F32)
            nc.gpsimd.partition_broadcast(abc[:], arow[:])
            iota_dst = constp.tile([P, KMAX * P], BF16)
            nc.gpsimd.iota(iota_dst[:], pattern=[[0, KMAX], [1, P]], base=0,
                           channel_multiplier=0,
                           allow_small_or_imprecise_dtypes=True)
            iota_gr = constp.tile([P, P], BF16)
            nc.gpsimd.iota(iota_gr[:], pattern=[[1, P]], base=0,
                           channel_multiplier=0,
                           allow_small_or_imprecise_dtypes=True)
            cntc = constp.tile([G, 1], F32)
            nc.sync.dma_start(out=cntc[:], in_=cnt_in[:])
            rcnt = constp.tile([G, 1], F32)
            nc.vector.tensor_scalar_max(rcnt[:], cntc[:], 1.0)
            nc.vector.reciprocal(rcnt[:], rcnt[:])

            # ---- resident metadata ----
            idx_t = metap.tile([P, NROWS // 16], I16)
            nc.sync.dma_start(out=idx_t[:], in_=idx_in[:])
            dst_t = metap.tile([P, CHKP], BF16)
            nc.sync.dma_start(out=dst_t[:], in_=dst_in[:])
            bat_t = metap.tile([P, TPC], BF16)
            nc.sync.dma_start(out=bat_t[:], in_=bat_in[:])
            dinv = metap.tile([P, TPC], F32)
            nc.sync.dma_start(out=dinv[:], in_=dinv_in[:])

            with tc.tile_pool(name="poolacc", bufs=1, space="PSUM") as pacc:
                pooled_ps = pacc.tile([G, D], F32)

                def phase(accum_pool):
                    with (
                        tc.tile_pool(name="xcp", bufs=3) as xcp,
                        tc.tile_pool(name="ohp", bufs=2) as ohp,
                        tc.tile_pool(name="accps", bufs=2,
                                     space="PSUM") as accps,
                        tc.tile_pool(name="accsb", bufs=2) as accsb,
                        tc.tile_pool(name="ups", bufs=2, space="PSUM") as ups,
                        tc.tile_pool(name="epi", bufs=2) as epip,
                        tc.tile_pool(name="sm", bufs=2) as smp,
                    ):
                        for g in range(NG):
                            t0 = g * TGRP
                            t1 = min(t0 + TGRP, TPC)
                            xt = xcp.tile([P, GW * D], XDT, tag="xt")
                            if mode == "tgather":
                                for ci in range(CPG):
                                    q = g * CPG + ci
                                    nc.gpsimd.dma_gather(
                                        xt[:, ci * TCH * D:
                                           (ci + 1) * TCH * D]
                                        .rearrange("p (j i) -> p j i", i=P),
                                        xcg_in[:],
                                        idx_t[:, q * 8:(q + 1) * 8],
                                        P, P, TEL,
                                        elem_step=TEL,
                                        transpose=True,
                                        single_packet=False)
                            elif mode == "gather":
                                for l0, k in calls:
                                    q0 = g * GP + l0
                                    nc.gpsimd.dma_gather(
                                        xt[:, l0 * PIECE * D:
                                           (l0 + k) * PIECE * D]
                                        .rearrange("p (k e) -> p k e", k=k),
                                        xcg_in[:],
                                        idx_t[:, q0 * 8:(q0 + k) * 8],
                                        k * P, k * P, PIECE * D,
                                        elem_step=PIECE * D,
                                        single_packet=False)
                            else:
                                eng = nc.sync if g % 2 == 0 else nc.scalar
                                eng.dma_start(
                                    out=xt[:].rearrange(
                                        "p (l e) -> p l e", l=GP),
                                    in_=xcg_in[g * GP * P:(g + 1) * GP * P, :]
                                    .rearrange("(l p) e -> p l e", p=P))
                            for t in range(t0, t1):
                                KEt = KE[t]
                                ob = int(CB[t] - CB[t0])  # chunk offset in xt
                                DB = g * GW + ob          # dstloc column base
                                oh = ohp.tile([P, KMAX * P], BF16, tag="oh")
                                nc.vector.tensor_tensor(
                                    out=oh[:, :KEt * P],
                                    in0=dst_t[:, DB:DB + KEt]
                                        .to_broadcast([P, KEt, P]),
                                    in1=iota_dst[:, :KEt * P],
                                    op=OP.is_equal)
                                acc_ps = accps.tile([P, P], F32, tag="acc")
                                for k in range(KEt):
                                    nc.tensor.matmul(
                                        out=acc_ps[:],
                                        lhsT=xt[:, (ob + k) * D:
                                                (ob + k + 1) * D],
                                        rhs=oh[:, k * P:(k + 1) * P],
                                        start=(k == 0), stop=(k == KEt - 1))
                                accT = accsb.tile([P, P], F32, tag="accT")
                                nc.scalar.copy(accT[:], acc_ps[:])
                                u_ps = ups.tile([P, D], F32, tag="u")
                                nc.tensor.matmul(out=u_ps[:], lhsT=accT[:],
                                                 rhs=w_t[:],
                                                 start=True, stop=True)
                                # epilogue
                                u = epip.tile([P, D], F32, tag="ue")
                                nc.vector.tensor_scalar_mul(
                                    u[:], u_ps[:], dinv[:, t:t + 1])
                                nc.vector.tensor_tensor(
                                    out=u[:], in0=u[:], in1=bbc[:], op=OP.add)
                                pos = epip.tile([P, D], F32, tag="pos")
                                nc.scalar.activation(pos[:], u[:], AF.Relu)
                                neg = epip.tile([P, D], F32, tag="neg")
                                nc.vector.tensor_tensor(
                                    out=neg[:], in0=u[:], in1=pos[:],
                                    op=OP.subtract)
                                nc.vector.tensor_tensor(
                                    out=neg[:], in0=neg[:], in1=abc[:],
                                    op=OP.mult)
                                v = epip.tile([P, D], F32, tag="v")
                                nc.vector.tensor_tensor(
                                    out=v[:], in0=pos[:], in1=neg[:],
                                    op=OP.add)
                                sq = epip.tile([P, D], F32, tag="sq")
                                ss = smp.tile([P, 1], F32, tag="ss")
                                nc.scalar.activation(sq[:], v[:], AF.Square,
                                                     accum_out=ss[:])
                                nc.scalar.sqrt(ss[:], ss[:])
                                nc.vector.tensor_scalar_max(ss[:], ss[:],
                                                            1e-12)
                                nc.vector.reciprocal(ss[:], ss[:])
                                o3 = epip.tile([P, D], F32, tag="o3")
                                nc.scalar.mul(o3[:], v[:], ss[:])
                                ohb = epip.tile([P, P], F32, tag="ohb")
                                nc.vector.tensor_tensor(
                                    out=ohb[:],
                                    in0=bat_t[:, t:t + 1].to_broadcast([P, P]),
                                    in1=iota_gr[:], op=OP.is_equal)
                                nc.tensor.matmul(out=accum_pool[:],
                                                 lhsT=ohb[:], rhs=o3[:],
                                                 start=(t == 0),
                                                 stop=(t == TPC - 1),
                                                 skip_group_check=True)

                if repeats > 1:
                    with tc.For_i(0, repeats, 1):
                        phase(pooled_ps)
                else:
                    phase(pooled_ps)

                with tc.tile_pool(name="fin", bufs=1) as finp:
                    pooled_sb = finp.tile([G, D], F32)
                    nc.vector.tensor_copy(pooled_sb[:], pooled_ps[:])
                    nc.sync.dma_start(out=ar_in[:], in_=pooled_sb[:])
                    nc.gpsimd.collective_compute(
                        "AllReduce", OP.add,
                        replica_groups=[list(range(NCORES))],
                        ins=[ar_in[:]], outs=[ar_out[:]],
                    )
                    red = finp.tile([G, D], F32)
                    nc.sync.dma_start(out=red[:], in_=ar_out[:])
                    fin = finp.tile([G, D], F32)
                    nc.scalar.mul(fin[:], red[:], rcnt[:])
                    nc.sync.dma_start(out=pooled_out[:], in_=fin[:])

    nc.compile()
    return nc


def make_in_maps(packed, W, b, prelu_a):
    W = np.ascontiguousarray(W, np.float32)
    b = np.ascontiguousarray(b, np.float32).reshape(1, D)
    a = np.ascontiguousarray(prelu_a, np.float32).reshape(1, D)
    return [
        {
            "xcg": packed["xcg"][c],
            "idx16": packed["idx16"],
            "dstloc": packed["dstloc"][c],
            "dinvsh": packed["dinvsh"][c],
            "batsh": packed["batsh"][c],
            "w": W, "b": b, "a": a, "cnt": packed["cnt"],
        }
        for c in range(NCORES)
    ]


def kernel(x, edge_index, batch, W, b, prelu_a):
    x = np.asarray(x)
    edge_index = np.asarray(edge_index)
    batch = np.asarray(batch)
    packed = pack_inputs(x, edge_index, batch)
    nc = build_program(packed["TPC"], packed["KE"], packed["GW"],
                       packed["NG"])
    in_maps = make_in_maps(packed, np.asarray(W), np.asarray(b),
                           np.asarray(prelu_a))
    res = run_bass_kernel_spmd(nc, in_maps, core_ids=list(range(NCORES)))
    return np.asarray(res.results[0]["pooled"], np.float32)


# revision 17
# speedup vs baseline: 3.0079x; 3.0079x over previous
"""GCNConv + PReLU + L2-normalize + global_mean_pool on 8 trn2 NeuronCores.

Strategy: edge-parallel with host-materialized messages.  All gather
indices are host-known, so instead of an on-device row gather (the
original design was bound by SWDGE gather-descriptor generation on
GPSIMD at ~5ns/256B-row), the host pre-gathers dinv[s]*x[s] for every
edge slot into a dense, destination-tile-grouped chunk stream.
Self-loops are ordinary edges (s == d) under the identity

  out[d] = dinv[d] * ( sum_{s->d, s incl. d} dinv[s]*x[s] ) @ W + b

because W is linear and factors out of the edge sum.  The device work
per destination tile t (128 nodes) is then:

  1. bulk-stream the tile group's chunk block Xc [128 slots, ~36KB]
  2. one-hot OH[slot, dst] built on DVE from packed dst locators
  3. KE scatter matmuls accumulating accT[feat, dst] in PSUM
     (lhsT = Xc chunk [slot, feat], rhs = OH chunk [slot, dst])
  4. one W matmul: u[dst, h] = accT^T @ W  (accT is already [feat, dst],
     exactly the lhsT the PE wants -- no transposes anywhere)
  5. epilogue: *dinv[d], +b, PReLU, L2-normalize, pooling matmul into a
     per-graph PSUM accumulator

Per-graph partial sums are AllReduced across the 8 cores and divided by
per-graph node counts.  No AllGather, no phase-1 x@W, no per-edge
indexed DMA.

DMA mechanism (measured): a plain dma_start drains its descriptors on a
single DMA engine (~21 GB/s/ring; only the SP and ACT HWDGE rings
exist), which would make the ~55 MB/core stream take ~2.6 ms.
dma_gather with single_packet=False instead spreads per-index packets
across all DMA engines, so the stream is fetched by dma_gather calls of
128 host-baked *sequential* indices, each index naming one ~36KB
[partition x tile-group] row of the host-packed stream: ~1.7us of
descriptor generation per ~4.6MB call, drain at the full HBM rate.

Nodes are assigned to (core, tile, partition) snake-balanced by
in-degree so per-tile chunk counts are uniform across cores (the chunk
count per tile is baked into the shared SPMD program as the max over
cores).  Padding slots carry zero rows and a dst locator of 255 which
never matches the 0..127 iota; padded node rows carry batch id 255 so
the pooling one-hot drops them.

Roofline: ~212k edge slots/core * 256B (bf16) = ~55 MB of sequential
HBM reads = ~155us at 358 GB/s, with PE (~110us) and DVE (~90us)
hidden underneath.
"""

import numpy as np
import ml_dtypes

import concourse.bacc as bacc
import concourse.tile as tile
import concourse.mybir as mybir
from concourse.bass_utils import run_bass_kernel_spmd

P = 128          # partitions / tile size
D = 128          # feature dim
G = 128          # number of graphs
NCORES = 8
TGRP = 8         # dst tiles per gather group
EB = 4096        # gather piece size in bytes (one DMA packet per index)
PIECE = EB // 256            # chunks per piece (bf16 chunk row = 256B)
MAXIDX = 8 * P   # max indices per dma_gather call (validated on HW)
MODE = "gather"             # "tgather" | "gather" | "dma"
TEB = 8192       # transpose-gather row bytes (-> 32-col x 128-part tile)
TCH = TEB // 256             # chunks covered per transpose-gather index
TEL = TEB // 2               # elements per transpose-gather row (bf16)

F32 = mybir.dt.float32
BF16 = mybir.dt.bfloat16
I16 = mybir.dt.int16
AF = mybir.ActivationFunctionType
OP = mybir.AluOpType

XDT = BF16
XDT_NP = ml_dtypes.bfloat16


# ----------------------------------------------------------------------------
# Host-side packing: node permutation, edge->slot layout, per-core arrays.
# ----------------------------------------------------------------------------
def pack_inputs(x, edge_index, batch):
    N, Dx = x.shape
    E = edge_index.shape[1]
    src0 = edge_index[0].astype(np.int64)
    dst0 = edge_index[1].astype(np.int64)

    ntiles = -(-(-(-N // P)) // NCORES) * NCORES
    TPC = ntiles // NCORES
    NPC = TPC * P
    NPAD = NCORES * NPC

    indeg = np.bincount(dst0, minlength=N).astype(np.int64)
    dinv = (1.0 / np.sqrt((indeg + 1).astype(np.float64))).astype(np.float32)

    # ---- assign nodes to tiles: snake over tiles in descending in-degree ----
    order = np.argsort(-indeg, kind="stable")
    nrounds = NPAD // ntiles
    tile_seq = np.arange(ntiles)
    snake = np.empty((nrounds, ntiles), np.int64)
    snake[0::2] = tile_seq
    snake[1::2] = tile_seq[::-1]
    tile_of_slot = snake.reshape(-1)          # [NPAD]
    p_of_slot = np.repeat(np.arange(nrounds), ntiles)
    node_of_slot = np.full(NPAD, -1, np.int64)
    node_of_slot[:N] = order

    load = np.zeros(ntiles, np.int64)
    np.add.at(load, tile_of_slot[:N], indeg[order] + 1)

    # ---- assign tiles to cores: snake over cores in descending load ----
    tord = np.argsort(-load, kind="stable")
    core_of_tile = np.empty(ntiles, np.int64)
    tidx_of_tile = np.empty(ntiles, np.int64)
    cseq = np.arange(NCORES)
    for r in range(TPC):
        cs = cseq if r % 2 == 0 else cseq[::-1]
        tr = tord[r * NCORES:(r + 1) * NCORES]
        core_of_tile[tr] = cs
        tidx_of_tile[tr] = r

    row_of_slot = (core_of_tile[tile_of_slot] * NPC
                   + tidx_of_tile[tile_of_slot] * P + p_of_slot)
    row_of_node = np.empty(N, np.int64)
    real = node_of_slot >= 0
    row_of_node[node_of_slot[real]] = row_of_slot[real]
    node_at_row = np.full(NPAD, -1, np.int64)
    node_at_row[row_of_slot] = node_of_slot

    # ---- edge slots (self-loops appended as ordinary edges) ----
    loop = np.arange(N, dtype=np.int64)
    src = np.concatenate([src0, loop])
    dst = np.concatenate([dst0, loop])
    er = row_of_node[dst]
    ec = er // NPC
    et = (er % NPC) // P
    ep = er % P
    gt = ec * TPC + et
    eo = np.argsort(gt, kind="stable")
    gts = gt[eo]
    srcs = src[eo]
    eps = ep[eo]

    mct = np.bincount(gt, minlength=ntiles).reshape(NCORES, TPC)
    KE = np.maximum(1, -(-mct.max(axis=0) // P)).astype(np.int64)  # [TPC]
    CB = np.concatenate([[0], np.cumsum(KE)]).astype(np.int64)

    # ---- gather groups of TGRP tiles, padded to uniform width GW chunks ----
    NG = -(-TPC // TGRP)
    gwidths = [int(CB[min((g + 1) * TGRP, TPC)] - CB[g * TGRP])
               for g in range(NG)]
    align = TCH if MODE == "tgather" else PIECE
    GW = -(-max(gwidths) // align) * align
    CHKP = NG * GW                       # padded chunk count

    starts = np.searchsorted(gts, np.arange(ntiles))
    j = np.arange(E + N) - starts[gts]
    slotp = j % P
    tloc = gts % TPC                     # tile index within core
    grp = tloc // TGRP
    # chunk column in the padded group-local layout
    colp = grp * GW + (CB[tloc] - CB[grp * TGRP]) + j // P
    core = gts // TPC

    xs = (np.asarray(x, np.float32) * dinv[:, None]).astype(XDT_NP)
    if MODE == "tgather":
        # table row (q*128 + i) is call q's index i; its TEL elements land
        # transposed as the [128-partition x (TCH*128-col)/128-col] tile:
        # elem[j*128 + c] -> stream[part c, call-base + j*128 + i]
        NQ = NG * (GW // TCH)
        NROWS = NQ * P
        assert NROWS <= 32767
        xth = np.zeros((NCORES, P, CHKP, Dx), XDT_NP)
        xth[core, slotp, colp] = xs[srcs]
        xcg = np.ascontiguousarray(
            xth.reshape(NCORES, P, NQ, TEL // P, P)
            .transpose(0, 2, 4, 3, 1)).reshape(NCORES, NROWS, TEL)
        del xth
    else:
        # xcg row (q*128 + p) is piece q (PIECE chunks) of partition p's
        # stream; piece q = g*GP + l covers group g's local chunks
        # [l*PIECE, (l+1)*PIECE)
        GP = GW // PIECE
        NROWS = NG * GP * P
        assert NROWS <= 32767
        lc = colp - grp * GW             # local chunk within group
        xcg = np.zeros((NCORES, NROWS, PIECE, Dx), XDT_NP)
        xcg[core, (grp * GP + lc // PIECE) * P + slotp,
            lc % PIECE] = xs[srcs]
        xcg = xcg.reshape(NCORES, NROWS, PIECE * Dx)
    dstloc = np.full((NCORES, P, CHKP), 255.0, ml_dtypes.bfloat16)
    dstloc[core, slotp, colp] = eps.astype(ml_dtypes.bfloat16)

    # gather indices are the sequence 0..NROWS-1 in the wrapped int16 layout
    # (value i lives at [i%16, i//16], replicated to 128 partitions)
    idx16 = np.zeros((16, NROWS // 16), np.int16)
    i = np.arange(NROWS)
    idx16[i % 16, i // 16] = i.astype(np.int16)
    idx16 = np.tile(idx16, (8, 1))

    # ---- per-core node arrays ----
    nar = node_at_row.reshape(NCORES, NPC)
    dinvsh = np.ones((NCORES, P, TPC), np.float32)
    batsh = np.full((NCORES, P, TPC), 255.0, np.float32)
    for c in range(NCORES):
        m = nar[c] >= 0
        dv = np.ones(NPC, np.float32)
        dv[m] = dinv[nar[c][m]]
        dinvsh[c] = dv.reshape(TPC, P).T
        bt = np.full(NPC, 255.0, np.float32)
        bt[m] = batch[nar[c][m]].astype(np.float32)
        batsh[c] = bt.reshape(TPC, P).T
    batsh = batsh.astype(ml_dtypes.bfloat16)

    cnt = np.bincount(np.asarray(batch, np.int64), minlength=G)
    cnt = cnt.astype(np.float32).reshape(G, 1)

    return dict(TPC=TPC, KE=KE, GW=GW, NG=NG,
                xcg=xcg, dstloc=dstloc, idx16=idx16,
                dinvsh=dinvsh, batsh=batsh, cnt=cnt)


# ----------------------------------------------------------------------------
# Device program.
# ----------------------------------------------------------------------------
def build_program(TPC, KE, GW, NG, repeats=1, mode=MODE):
    KE = [int(k) for k in KE]
    CB = np.concatenate([[0], np.cumsum(KE)]).astype(np.int64)
    KMAX = max(KE)
    CHKP = NG * GW
    if mode == "tgather":
        CPG = GW // TCH                  # transpose-gather calls per group
        NROWS = NG * CPG * P
        ROWEL = TEL
    else:
        GP = GW // PIECE
        NROWS = NG * GP * P
        ROWEL = PIECE * D
        # per-group gather calls: (local piece base, piece count)
        calls = [(l0, min(MAXIDX // P, GP - l0))
                 for l0 in range(0, GP, MAXIDX // P)]

    nc = bacc.Bacc("TRN2", target_bir_lowering=False, debug=False,
                   num_devices=NCORES)

    xcg_in = nc.dram_tensor("xcg", [NROWS, ROWEL], XDT,
                            kind="ExternalInput")
    idx_in = nc.dram_tensor("idx16", [P, NROWS // 16], I16,
                            kind="ExternalInput")
    dst_in = nc.dram_tensor("dstloc", [P, CHKP], BF16, kind="ExternalInput")
    dinv_in = nc.dram_tensor("dinvsh", [P, TPC], F32, kind="ExternalInput")
    bat_in = nc.dram_tensor("batsh", [P, TPC], BF16, kind="ExternalInput")
    w_in = nc.dram_tensor("w", [D, D], F32, kind="ExternalInput")
    b_in = nc.dram_tensor("b", [1, D], F32, kind="ExternalInput")
    a_in = nc.dram_tensor("a", [1, D], F32, kind="ExternalInput")
    cnt_in = nc.dram_tensor("cnt", [G, 1], F32, kind="ExternalInput")
    pooled_out = nc.dram_tensor("pooled", [G, D], F32, kind="ExternalOutput")

    ar_in = nc.dram_tensor("ar_in", [G, D], F32)
    ar_out = nc.dram_tensor("ar_out", [G, D], F32, addr_space="Shared")

    with tile.TileContext(nc, num_cores=NCORES) as tc:
        with (
            tc.tile_pool(name="const", bufs=1) as constp,
            tc.tile_pool(name="meta", bufs=1) as metap,
        ):
            # ---- constants ----
            w_t = constp.tile([D, D], F32)
            nc.sync.dma_start(out=w_t[:], in_=w_in[:])
            brow = constp.tile([1, D], F32)
            nc.sync.dma_start(out=brow[:], in_=b_in[:])
            bbc = constp.tile([P, D], F32)
            nc.gpsimd.partition_broadcast(bbc[:], brow[:])
            arow = constp.tile([1, D], F32)
            nc.sync.dma_start(out=arow[:], in_=a_in[:])
            abc = constp.tile([P, D], F32)
            nc.gpsimd.partition_broadcast(abc[:], arow[:])
            iota_dst = constp.tile([P, KMAX * P], BF16)
            nc.gpsimd.iota(iota_dst[:], pattern=[[0, KMAX], [1, P]], base=0,
                           channel_multiplier=0,
                           allow_small_or_imprecise_dtypes=True)
            iota_gr = constp.tile([P, P], BF16)
            nc.gpsimd.iota(iota_gr[:], pattern=[[1, P]], base=0,
                           channel_multiplier=0,
                           allow_small_or_imprecise_dtypes=True)
            cntc = constp.tile([G, 1], F32)
            nc.sync.dma_start(out=cntc[:], in_=cnt_in[:])
            rcnt = constp.tile([G, 1], F32)
            nc.vector.tensor_scalar_max(rcnt[:], cntc[:], 1.0)
            nc.vector.reciprocal(rcnt[:], rcnt[:])

            # ---- resident metadata ----
            idx_t = metap.tile([P, NROWS // 16], I16)
            nc.sync.dma_start(out=idx_t[:], in_=idx_in[:])
            dst_t = metap.tile([P, CHKP], BF16)
            nc.sync.dma_start(out=dst_t[:], in_=dst_in[:])
            bat_t = metap.tile([P, TPC], BF16)
            nc.sync.dma_start(out=bat_t[:], in_=bat_in[:])
            dinv = metap.tile([P, TPC], F32)
            nc.sync.dma_start(out=dinv[:], in_=dinv_in[:])

            with tc.tile_pool(name="poolacc", bufs=1, space="PSUM") as pacc:
                pooled_ps = pacc.tile([G, D], F32)

                def phase(accum_pool):
                    with (
                        tc.tile_pool(name="xcp", bufs=3) as xcp,
                        tc.tile_pool(name="ohp", bufs=2) as ohp,
                        tc.tile_pool(name="accps", bufs=2,
                                     space="PSUM") as accps,
                        tc.tile_pool(name="accsb", bufs=2) as accsb,
                        tc.tile_pool(name="ups", bufs=2, space="PSUM") as ups,
                        tc.tile_pool(name="epi", bufs=2) as epip,
                        tc.tile_pool(name="sm", bufs=2) as smp,
                    ):
                        for g in range(NG):
                            t0 = g * TGRP
                            t1 = min(t0 + TGRP, TPC)
                            xt = xcp.tile([P, GW * D], XDT, tag="xt")
                            if mode == "tgather":
                                for ci in range(CPG):
                                    q = g * CPG + ci
                                    nc.gpsimd.dma_gather(
                                        xt[:, ci * TCH * D:
                                           (ci + 1) * TCH * D]
                                        .rearrange("p (j i) -> p j i", i=P),
                                        xcg_in[:],
                                        idx_t[:, q * 8:(q + 1) * 8],
                                        P, P, TEL,
                                        elem_step=TEL,
                                        transpose=True,
                                        single_packet=False)
                            elif mode == "gather":
                                for l0, k in calls:
                                    q0 = g * GP + l0
                                    nc.gpsimd.dma_gather(
                                        xt[:, l0 * PIECE * D:
                                           (l0 + k) * PIECE * D]
                                        .rearrange("p (k e) -> p k e", k=k),
                                        xcg_in[:],
                                        idx_t[:, q0 * 8:(q0 + k) * 8],
                                        k * P, k * P, PIECE * D,
                                        elem_step=PIECE * D,
                                        single_packet=False)
                            else:
                                eng = nc.sync if g % 2 == 0 else nc.scalar
                                eng.dma_start(
                                    out=xt[:].rearrange(
                                        "p (l e) -> p l e", l=GP),
                                    in_=xcg_in[g * GP * P:(g + 1) * GP * P, :]
                                    .rearrange("(l p) e -> p l e", p=P))
                            for t in range(t0, t1):
                                KEt = KE[t]
                                ob = int(CB[t] - CB[t0])  # chunk offset in xt
                                DB = g * GW + ob          # dstloc column base
                                oh = ohp.tile([P, KMAX * P], BF16, tag="oh")
                                nc.vector.tensor_tensor(
                                    out=oh[:, :KEt * P],
                                    in0=dst_t[:, DB:DB + KEt]
                                        .to_broadcast([P, KEt, P]),
                                    in1=iota_dst[:, :KEt * P],
                                    op=OP.is_equal)
                                acc_ps = accps.tile([P, P], F32, tag="acc")
                                for k in range(KEt):
                                    nc.tensor.matmul(
                                        out=acc_ps[:],
                                        lhsT=xt[:, (ob + k) * D:
                                                (ob + k + 1) * D],
                                        rhs=oh[:, k * P:(k + 1) * P],
                                        start=(k == 0), stop=(k == KEt - 1))
                                accT = accsb.tile([P, P], F32, tag="accT")
                                nc.scalar.copy(accT[:], acc_ps[:])
                                u_ps = ups.tile([P, D], F32, tag="u")
                                nc.tensor.matmul(out=u_ps[:], lhsT=accT[:],
                                                 rhs=w_t[:],
                                                 start=True, stop=True)
                                # epilogue
                                u = epip.tile([P, D], F32, tag="ue")
                                nc.vector.tensor_scalar_mul(
                                    u[:], u_ps[:], dinv[:, t:t + 1])
                                nc.vector.tensor_tensor(
                                    out=u[:], in0=u[:], in1=bbc[:], op=OP.add)
                                pos = epip.tile([P, D], F32, tag="pos")
                                nc.scalar.activation(pos[:], u[:], AF.Relu)
                                neg = epip.tile([P, D], F32, tag="neg")
                                nc.vector.tensor_tensor(
                                    out=neg[:], in0=u[:], in1=pos[:],
                                    op=OP.subtract)
                                nc.vector.tensor_tensor(
                                    out=neg[:], in0=neg[:], in1=abc[:],
                                    op=OP.mult)
                                v = epip.tile([P, D], F32, tag="v")
                                nc.vector.tensor_tensor(
                                    out=v[:], in0=pos[:], in1=neg[:],
                                    op=OP.add)
                                sq = epip.tile([P, D], F32, tag="sq")
                                ss = smp.tile([P, 1], F32, tag="ss")
                                nc.scalar.activation(sq[:], v[:], AF.Square,
                                                     accum_out=ss[:])
                                nc.scalar.sqrt(ss[:], ss[:])
                                nc.vector.tensor_scalar_max(ss[:], ss[:],
                                                            1e-12)
                                nc.vector.reciprocal(ss[:], ss[:])
                                o3 = epip.tile([P, D], F32, tag="o3")
                                nc.scalar.mul(o3[:], v[:], ss[:])
                                ohb = epip.tile([P, P], F32, tag="ohb")
                                nc.vector.tensor_tensor(
                                    out=ohb[:],
                                    in0=bat_t[:, t:t + 1].to_broadcast([P, P]),
                                    in1=iota_gr[:], op=OP.is_equal)
                                nc.tensor.matmul(out=accum_pool[:],
                                                 lhsT=ohb[:], rhs=o3[:],
                                                 start=(t == 0),
                                                 stop=(t == TPC - 1),
                                                 skip_group_check=True)

                if repeats > 1:
                    with tc.For_i(0, repeats, 1):
                        phase(pooled_ps)
                else:
                    phase(pooled_ps)

                with tc.tile_pool(name="fin", bufs=1) as finp:
                    pooled_sb = finp.tile([G, D], F32)
                    nc.vector.tensor_copy(pooled_sb[:], pooled_ps[:])
                    nc.sync.dma_start(out=ar_in[:], in_=pooled_sb[:])
                    nc.gpsimd.collective_compute(
                        "AllReduce", OP.add,
                        replica_groups=[list(range(NCORES))],
                        ins=[ar_in[:]], outs=[ar_out[:]],
                    )
                    red = finp.tile([G, D], F32)
                    nc.sync.dma_start(out=red[:], in_=ar_out[:])
                    fin = finp.tile([G, D], F32)
                    nc.scalar.mul(fin[:], red[:], rcnt[:])
                    nc.sync.dma_start(out=pooled_out[:], in_=fin[:])

    nc.compile()
    return nc


def make_in_maps(packed, W, b, prelu_a):
    W = np.ascontiguousarray(W, np.float32)
    b = np.ascontiguousarray(b, np.float32).reshape(1, D)
    a = np.ascontiguousarray(prelu_a, np.float32).reshape(1, D)
    return [
        {
            "xcg": packed["xcg"][c],
            "idx16": packed["idx16"],
            "dstloc": packed["dstloc"][c],
            "dinvsh": packed["dinvsh"][c],
            "batsh": packed["batsh"][c],
            "w": W, "b": b, "a": a, "cnt": packed["cnt"],
        }
        for c in range(NCORES)
    ]


def kernel(x, edge_index, batch, W, b, prelu_a):
    x = np.asarray(x)
    edge_index = np.asarray(edge_index)
    batch = np.asarray(batch)
    packed = pack_inputs(x, edge_index, batch)
    nc = build_program(packed["TPC"], packed["KE"], packed["GW"],
                       packed["NG"])
    in_maps = make_in_maps(packed, np.asarray(W), np.asarray(b),
                           np.asarray(prelu_a))
    res = run_bass_kernel_spmd(nc, in_maps, core_ids=list(range(NCORES)))
    return np.asarray(res.results[0]["pooled"], np.float32)


# revision 18
# speedup vs baseline: 20.4273x; 6.7912x over previous
"""GCNConv + PReLU + L2-normalize + global_mean_pool on 8 trn2 NeuronCores.

Strategy: edge-parallel with host-materialized messages.  All gather
indices are host-known, so the host pre-gathers dinv[s]*x[s] for every
edge slot into a dense, destination-tile-grouped chunk stream (fp8).
Self-loops are ordinary edges (s == d) under the identity

  out[d] = dinv[d] * ( sum_{s->d, s incl. d} dinv[s]*x[s] ) @ W + b

because W is linear and factors out of the edge sum.  The device work
per destination tile t (128 nodes):

  1. bulk-stream the tile group's chunk block Xc [128 slots, GW chunks]
  2. one-hot OH[slot, dst] built on DVE from packed dst locators
  3. KE scatter matmuls accumulating accT[feat, dst] in PSUM
     (lhsT = Xc chunk [slot, feat], rhs = OH chunk [slot, dst])
  4. one W matmul: u[dst, h] = accT^T @ W  (accT is already [feat, dst],
     exactly the lhsT the PE wants -- no transposes anywhere)
  5. epilogue: *dinv[d], +b, PReLU, L2-normalize, pooling matmul into a
     per-graph PSUM accumulator

Per-graph partial sums are AllReduced across the 8 cores and divided by
per-graph node counts.

Stream ingest (all rates measured on this hardware): a dynamic-queue
dma_start drains its descriptors serially at ~21 GB/s per HWDGE ring
(only SP and ACT rings exist); dma_gather with single_packet=False
spreads packets across DMA engines but collapses above 2KB packets
(4KB pieces: 4.2 GB/s; 36KB rows: 4.3 GB/s; transpose-mode 256B xbar
sprays: 1.4 GB/s).  The fast configuration is dma_gather over 2KB
pieces (~43 GB/s) -- read directly out of the partition-line table via
an access-pattern view, indices host-baked -- COMBINED with big-line
dma_start groups on the SP and ACT rings (~21 GB/s each).  Groups
round-robin over the three mechanisms (MECHS) so the three drains run
concurrently: ~85 GB/s aggregate, on an fp8 stream half the size.

Nodes are assigned to (core, tile, partition) snake-balanced by
in-degree so per-tile chunk counts are uniform across cores (the chunk
count per tile is baked into the shared SPMD program as the max over
cores).  Padding slots carry zero rows and a dst locator of 255 which
never matches the 0..127 iota; padded node rows carry batch id 255 so
the pooling one-hot drops them.
"""

import numpy as np
import ml_dtypes

import concourse.bacc as bacc
import concourse.tile as tile
import concourse.mybir as mybir
from concourse.bass_utils import run_bass_kernel_spmd

P = 128          # partitions / tile size
D = 128          # feature dim
G = 128          # number of graphs
NCORES = 8
TGRP = 8         # dst tiles per stream group
EB = 2048        # gather piece bytes (one DMA packet per index; >2KB is slow)
MAXP = 8         # max pieces per dma_gather call (1024 indices, HW-validated)
MECHS = ("g", "sp", "g", "act")   # per-group ingest mechanism cycle

F32 = mybir.dt.float32
BF16 = mybir.dt.bfloat16
I16 = mybir.dt.int16
AF = mybir.ActivationFunctionType
OP = mybir.AluOpType

XDT = mybir.dt.float8e4
XDT_NP = ml_dtypes.float8_e4m3
ESIZE = np.dtype(XDT_NP).itemsize
EBE = EB // ESIZE            # gather piece elements
PIECE = EB // (D * ESIZE)    # chunks per gather piece


# ----------------------------------------------------------------------------
# Host-side packing: node permutation, edge->slot layout, per-core arrays.
# ----------------------------------------------------------------------------
def pack_inputs(x, edge_index, batch):
    N, Dx = x.shape
    E = edge_index.shape[1]
    src0 = edge_index[0].astype(np.int64)
    dst0 = edge_index[1].astype(np.int64)

    ntiles = -(-(-(-N // P)) // NCORES) * NCORES
    TPC = ntiles // NCORES
    NPC = TPC * P
    NPAD = NCORES * NPC

    indeg = np.bincount(dst0, minlength=N).astype(np.int64)
    dinv = (1.0 / np.sqrt((indeg + 1).astype(np.float64))).astype(np.float32)

    # ---- assign nodes to tiles: snake over tiles in descending in-degree ----
    order = np.argsort(-indeg, kind="stable")
    nrounds = NPAD // ntiles
    tile_seq = np.arange(ntiles)
    snake = np.empty((nrounds, ntiles), np.int64)
    snake[0::2] = tile_seq
    snake[1::2] = tile_seq[::-1]
    tile_of_slot = snake.reshape(-1)          # [NPAD]
    p_of_slot = np.repeat(np.arange(nrounds), ntiles)
    node_of_slot = np.full(NPAD, -1, np.int64)
    node_of_slot[:N] = order

    load = np.zeros(ntiles, np.int64)
    np.add.at(load, tile_of_slot[:N], indeg[order] + 1)

    # ---- assign tiles to cores: snake over cores in descending load ----
    tord = np.argsort(-load, kind="stable")
    core_of_tile = np.empty(ntiles, np.int64)
    tidx_of_tile = np.empty(ntiles, np.int64)
    cseq = np.arange(NCORES)
    for r in range(TPC):
        cs = cseq if r % 2 == 0 else cseq[::-1]
        tr = tord[r * NCORES:(r + 1) * NCORES]
        core_of_tile[tr] = cs
        tidx_of_tile[tr] = r

    row_of_slot = (core_of_tile[tile_of_slot] * NPC
                   + tidx_of_tile[tile_of_slot] * P + p_of_slot)
    row_of_node = np.empty(N, np.int64)
    real = node_of_slot >= 0
    row_of_node[node_of_slot[real]] = row_of_slot[real]
    node_at_row = np.full(NPAD, -1, np.int64)
    node_at_row[row_of_slot] = node_of_slot

    # ---- edge slots (self-loops appended as ordinary edges) ----
    loop = np.arange(N, dtype=np.int64)
    src = np.concatenate([src0, loop])
    dst = np.concatenate([dst0, loop])
    er = row_of_node[dst]
    ec = er // NPC
    et = (er % NPC) // P
    ep = er % P
    gt = ec * TPC + et
    eo = np.argsort(gt, kind="stable")
    gts = gt[eo]
    srcs = src[eo]
    eps = ep[eo]

    mct = np.bincount(gt, minlength=ntiles).reshape(NCORES, TPC)
    KE = np.maximum(1, -(-mct.max(axis=0) // P)).astype(np.int64)  # [TPC]
    CB = np.concatenate([[0], np.cumsum(KE)]).astype(np.int64)

    # ---- stream groups of TGRP tiles, padded to uniform width GW chunks ----
    NG = -(-TPC // TGRP)
    gwidths = [int(CB[min((g + 1) * TGRP, TPC)] - CB[g * TGRP])
               for g in range(NG)]
    GW = -(-max(gwidths) // PIECE) * PIECE   # multiple of the piece size
    CHKP = NG * GW                       # padded chunk count

    starts = np.searchsorted(gts, np.arange(ntiles))
    j = np.arange(E + N) - starts[gts]
    slotp = j % P
    tloc = gts % TPC                     # tile index within core
    grp = tloc // TGRP
    # chunk column in the padded group-local layout
    colp = grp * GW + (CB[tloc] - CB[grp * TGRP]) + j // P
    core = gts // TPC

    # partition-line stream: xcg[c, p, col, :] = scaled source row
    xs = (np.asarray(x, np.float32) * dinv[:, None]).astype(XDT_NP)
    xcg = np.zeros((NCORES, P, CHKP, Dx), XDT_NP)
    xcg[core, slotp, colp] = xs[srcs]
    dstloc = np.full((NCORES, P, CHKP), 255.0, ml_dtypes.bfloat16)
    dstloc[core, slotp, colp] = eps.astype(ml_dtypes.bfloat16)

    # ---- gather indices over the piece view [(p q) e] of the line table ----
    GPG = GW // PIECE                    # pieces per partition per group
    NQP = NG * GPG                       # pieces per partition
    NROWS = P * NQP
    assert NROWS <= 32767
    idx16 = np.zeros((16, NROWS // 16), np.int16)
    cb = 0
    for g in range(NG):
        for l0 in range(0, GPG, MAXP):
            k = min(MAXP, GPG - l0)
            i = np.arange(k * P)
            vals = (i % P) * NQP + g * GPG + l0 + i // P
            idx16[i % 16, cb + i // 16] = vals.astype(np.int16)
            cb += k * 8
    idx16 = np.tile(idx16, (8, 1))

    # ---- per-core node arrays ----
    nar = node_at_row.reshape(NCORES, NPC)
    dinvsh = np.ones((NCORES, P, TPC), np.float32)
    batsh = np.full((NCORES, P, TPC), 255.0, np.float32)
    for c in range(NCORES):
        m = nar[c] >= 0
        dv = np.ones(NPC, np.float32)
        dv[m] = dinv[nar[c][m]]
        dinvsh[c] = dv.reshape(TPC, P).T
        bt = np.full(NPC, 255.0, np.float32)
        bt[m] = batch[nar[c][m]].astype(np.float32)
        batsh[c] = bt.reshape(TPC, P).T
    batsh = batsh.astype(ml_dtypes.bfloat16)

    cnt = np.bincount(np.asarray(batch, np.int64), minlength=G)
    cnt = cnt.astype(np.float32).reshape(G, 1)

    return dict(TPC=TPC, KE=KE, GW=GW, NG=NG,
                xcg=xcg.reshape(NCORES, P, CHKP * Dx),
                dstloc=dstloc, idx16=idx16,
                dinvsh=dinvsh, batsh=batsh, cnt=cnt)


# ----------------------------------------------------------------------------
# Device program.
# ----------------------------------------------------------------------------
def build_program(TPC, KE, GW, NG, repeats=1, mechs=MECHS):
    KE = [int(k) for k in KE]
    CB = np.concatenate([[0], np.cumsum(KE)]).astype(np.int64)
    KMAX = max(KE)
    CHKP = NG * GW
    GPG = GW // PIECE
    NQP = NG * GPG
    NROWS = P * NQP

    nc = bacc.Bacc("TRN2", target_bir_lowering=False, debug=False,
                   num_devices=NCORES)

    xcg_in = nc.dram_tensor("xcg", [P, CHKP * D], XDT, kind="ExternalInput")
    idx_in = nc.dram_tensor("idx16", [P, NROWS // 16], I16,
                            kind="ExternalInput")
    dst_in = nc.dram_tensor("dstloc", [P, CHKP], BF16, kind="ExternalInput")
    dinv_in = nc.dram_tensor("dinvsh", [P, TPC], F32, kind="ExternalInput")
    bat_in = nc.dram_tensor("batsh", [P, TPC], BF16, kind="ExternalInput")
    w_in = nc.dram_tensor("w", [D, D], F32, kind="ExternalInput")
    b_in = nc.dram_tensor("b", [1, D], F32, kind="ExternalInput")
    a_in = nc.dram_tensor("a", [1, D], F32, kind="ExternalInput")
    cnt_in = nc.dram_tensor("cnt", [G, 1], F32, kind="ExternalInput")
    pooled_out = nc.dram_tensor("pooled", [G, D], F32, kind="ExternalOutput")

    ar_in = nc.dram_tensor("ar_in", [G, D], F32)
    ar_out = nc.dram_tensor("ar_out", [G, D], F32, addr_space="Shared")

    with tile.TileContext(nc, num_cores=NCORES) as tc:
        with (
            tc.tile_pool(name="const", bufs=1) as constp,
            tc.tile_pool(name="meta", bufs=1) as metap,
        ):
            # ---- constants ----
            w_t = constp.tile([D, D], F32)
            nc.sync.dma_start(out=w_t[:], in_=w_in[:])
            brow = constp.tile([1, D], F32)
            nc.sync.dma_start(out=brow[:], in_=b_in[:])
            bbc = constp.tile([P, D], F32)
            nc.gpsimd.partition_broadcast(bbc[:], brow[:])
            arow = constp.tile([1, D], F32)
            nc.sync.dma_start(out=arow[:], in_=a_in[:])
            abc = constp.tile([P, D], F32)
            nc.gpsimd.partition_broadcast(abc[:], arow[:])
            iota_dst = constp.tile([P, KMAX * P], BF16)
            nc.gpsimd.iota(iota_dst[:], pattern=[[0, KMAX], [1, P]], base=0,
                           channel_multiplier=0,
                           allow_small_or_imprecise_dtypes=True)
            iota_gr = constp.tile([P, P], BF16)
            nc.gpsimd.iota(iota_gr[:], pattern=[[1, P]], base=0,
                           channel_multiplier=0,
                           allow_small_or_imprecise_dtypes=True)
            cntc = constp.tile([G, 1], F32)
            nc.sync.dma_start(out=cntc[:], in_=cnt_in[:])
            rcnt = constp.tile([G, 1], F32)
            nc.vector.tensor_scalar_max(rcnt[:], cntc[:], 1.0)
            nc.vector.reciprocal(rcnt[:], rcnt[:])

            # ---- resident metadata ----
            idx_t = metap.tile([P, NROWS // 16], I16)
            nc.sync.dma_start(out=idx_t[:], in_=idx_in[:])
            dst_t = metap.tile([P, CHKP], BF16)
            nc.sync.dma_start(out=dst_t[:], in_=dst_in[:])
            bat_t = metap.tile([P, TPC], BF16)
            nc.sync.dma_start(out=bat_t[:], in_=bat_in[:])
            dinv = metap.tile([P, TPC], F32)
            nc.sync.dma_start(out=dinv[:], in_=dinv_in[:])

            with tc.tile_pool(name="poolacc", bufs=1, space="PSUM") as pacc:
                pooled_ps = pacc.tile([G, D], F32)

                def phase(accum_pool):
                    with (
                        tc.tile_pool(name="xcp", bufs=3) as xcp,
                        tc.tile_pool(name="ohp", bufs=2) as ohp,
                        tc.tile_pool(name="accps", bufs=2,
                                     space="PSUM") as accps,
                        tc.tile_pool(name="accsb", bufs=2) as accsb,
                        tc.tile_pool(name="ups", bufs=2, space="PSUM") as ups,
                        tc.tile_pool(name="epi", bufs=2) as epip,
                        tc.tile_pool(name="sm", bufs=2) as smp,
                    ):
                        for g in range(NG):
                            t0 = g * TGRP
                            t1 = min(t0 + TGRP, TPC)
                            mech = mechs[g % len(mechs)]
                            xt = xcp.tile([P, GW * D], XDT, tag="xt")
                            if mech == "g":
                                for l0 in range(0, GPG, MAXP):
                                    k = min(MAXP, GPG - l0)
                                    cb = (g * GPG + l0) * 8
                                    nc.gpsimd.dma_gather(
                                        xt[:, l0 * PIECE * D:
                                           (l0 + k) * PIECE * D]
                                        .rearrange("p (k e) -> p k e", k=k),
                                        xcg_in[:].rearrange(
                                            "p (q e) -> (p q) e", e=EBE),
                                        idx_t[:, cb:cb + k * 8],
                                        k * P, k * P, EBE,
                                        elem_step=EBE,
                                        single_packet=False)
                            else:
                                eng = nc.sync if mech == "sp" else nc.scalar
                                eng.dma_start(
                                    out=xt[:],
                                    in_=xcg_in[:, g * GW * D:
                                               (g + 1) * GW * D])
                            for t in range(t0, t1):
                                KEt = KE[t]
                                ob = int(CB[t] - CB[t0])  # chunk offset in xt
                                DB = g * GW + ob          # dstloc column base
                                oh = ohp.tile([P, KMAX * P], XDT, tag="oh")
                                nc.vector.tensor_tensor(
                                    out=oh[:, :KEt * P],
                                    in0=dst_t[:, DB:DB + KEt]
                                        .to_broadcast([P, KEt, P]),
                                    in1=iota_dst[:, :KEt * P],
                                    op=OP.is_equal)
                                acc_ps = accps.tile([P, P], F32, tag="acc")
                                for k in range(KEt):
                                    nc.tensor.matmul(
                                        out=acc_ps[:],
                                        lhsT=xt[:, (ob + k) * D:
                                                (ob + k + 1) * D],
                                        rhs=oh[:, k * P:(k + 1) * P],
                                        start=(k == 0), stop=(k == KEt - 1))
                                accT = accsb.tile([P, P], F32, tag="accT")
                                nc.scalar.copy(accT[:], acc_ps[:])
                                u_ps = ups.tile([P, D], F32, tag="u")
                                nc.tensor.matmul(out=u_ps[:], lhsT=accT[:],
                                                 rhs=w_t[:],
                                                 start=True, stop=True)
                                # epilogue
                                u = epip.tile([P, D], F32, tag="ue")
                                nc.vector.tensor_scalar_mul(
                                    u[:], u_ps[:], dinv[:, t:t + 1])
                                nc.vector.tensor_tensor(
                                    out=u[:], in0=u[:], in1=bbc[:], op=OP.add)
                                pos = epip.tile([P, D], F32, tag="pos")
                                nc.scalar.activation(pos[:], u[:], AF.Relu)
                                neg = epip.tile([P, D], F32, tag="neg")
                                nc.vector.tensor_tensor(
                                    out=neg[:], in0=u[:], in1=pos[:],
                                    op=OP.subtract)
                                nc.vector.tensor_tensor(
                                    out=neg[:], in0=neg[:], in1=abc[:],
                                    op=OP.mult)
                                v = epip.tile([P, D], F32, tag="v")
                                nc.vector.tensor_tensor(
                                    out=v[:], in0=pos[:], in1=neg[:],
                                    op=OP.add)
                                sq = epip.tile([P, D], F32, tag="sq")
                                ss = smp.tile([P, 1], F32, tag="ss")
                                nc.scalar.activation(sq[:], v[:], AF.Square,
                                                     accum_out=ss[:])
                                nc.scalar.sqrt(ss[:], ss[:])
                                nc.vector.tensor_scalar_max(ss[:], ss[:],
                                                            1e-12)
                                nc.vector.reciprocal(ss[:], ss[:])
                                o3 = epip.tile([P, D], F32, tag="o3")
                                nc.scalar.mul(o3[:], v[:], ss[:])
                                ohb = epip.tile([P, P], F32, tag="ohb")
                                nc.vector.tensor_tensor(
                                    out=ohb[:],
                                    in0=bat_t[:, t:t + 1].to_broadcast([P, P]),
                                    in1=iota_gr[:], op=OP.is_equal)
                                nc.tensor.matmul(out=accum_pool[:],
                                                 lhsT=ohb[:], rhs=o3[:],
                                                 start=(t == 0),
                                                 stop=(t == TPC - 1),
                                                 skip_group_check=True)

                if repeats > 1:
                    with tc.For_i(0, repeats, 1):
                        phase(pooled_ps)
                else:
                    phase(pooled_ps)

                with tc.tile_pool(name="fin", bufs=1) as finp:
                    pooled_sb = finp.tile([G, D], F32)
                    nc.vector.tensor_copy(pooled_sb[:], pooled_ps[:])
                    nc.sync.dma_start(out=ar_in[:], in_=pooled_sb[:])
                    nc.gpsimd.collective_compute(
                        "AllReduce", OP.add,
                        replica_groups=[list(range(NCORES))],
                        ins=[ar_in[:]], outs=[ar_out[:]],
                    )
                    red = finp.tile([G, D], F32)
                    nc.sync.dma_start(out=red[:], in_=ar_out[:])
                    fin = finp.tile([G, D], F32)
                    nc.scalar.mul(fin[:], red[:], rcnt[:])
                    nc.sync.dma_start(out=pooled_out[:], in_=fin[:])

    nc.compile()
    return nc


def make_in_maps(packed, W, b, prelu_a):
    W = np.ascontiguousarray(W, np.float32)
    b = np.ascontiguousarray(b, np.float32).reshape(1, D)
    a = np.ascontiguousarray(prelu_a, np.float32).reshape(1, D)
    return [
        {
            "xcg": packed["xcg"][c],
            "idx16": packed["idx16"],
            "dstloc": packed["dstloc"][c],
            "dinvsh": packed["dinvsh"][c],
            "batsh": packed["batsh"][c],
            "w": W, "b": b, "a": a, "cnt": packed["cnt"],
        }
        for c in range(NCORES)
    ]


def kernel(x, edge_index, batch, W, b, prelu_a):
    x = np.asarray(x)
    edge_index = np.asarray(edge_index)
    batch = np.asarray(batch)
    packed = pack_inputs(x, edge_index, batch)
    nc = build_program(packed["TPC"], packed["KE"], packed["GW"],
                       packed["NG"])
    in_maps = make_in_maps(packed, np.asarray(W), np.asarray(b),
                           np.asarray(prelu_a))
    res = run_bass_kernel_spmd(nc, in_maps, core_ids=list(range(NCORES)))
    return np.asarray(res.results[0]["pooled"], np.float32)
